# revision 5
# baseline (speedup 1.0000x reference)
"""Trainium2 Bass kernel for 4-head spatial attention score softmax.

Reference computation:
    qk = einsum('bcxy,oc->boxy', fmap[1,256,64,64], W_qk[1024,256])
    q, k = split(qk, 2, axis=1)             # each [1, 512, 64, 64]
    q = q reshaped to heads, scaled by 128^-0.5
    sim[b,h,xy,uv] = q . k  (contraction over dim_head=128)
    out = softmax(sim, axis=-1)             # [1, 4, 4096, 4096] f32

Sharding: 8 cores = 4 heads x 2 query-halves. Each core projects q for its
2048 query columns + k for all 4096 columns (both via PE matmuls over the
channel dim), computes scores with f32r (FP22) matmuls, softmax
(exp on ScalarE with accumulated row sums, normalize on VectorE), and
streams its [2048, 4096] f32 output slab to HBM.
"""

import numpy as np

import concourse.bacc as bacc
import concourse.mybir as mybir
import concourse.tile as tile
from concourse import bass_utils

HEADS = 4
DIM_HEAD = 128
C = 256          # input channels
XY = 4096        # 64*64 spatial positions
QCHUNK = 2048    # query positions per core
N_CORES = 8
SCALE = DIM_HEAD ** -0.5

F32 = mybir.dt.float32
F32R = mybir.dt.float32r
BF16 = mybir.dt.bfloat16

# dtype of the q/k operands of the big score matmuls. 16-bit halves the PE
# streaming cost vs f32r (4-byte moving operand streams at ~2 cyc/elem) and
# enables fast weight load. fp16 over bf16: q/k are O(1), so the e5m10
# mantissa (exact inside the PE's FP22) cuts quantization error ~8x.
QK_DT = mybir.dt.float16


def _emit(tc, fmap_k, fmap_q, wqt, wkt, out):
    nc = tc.nc

    with tc.tile_pool(name="consts", bufs=1) as consts:
        # Weights transposed on host: [c, d] with c split into 2 partition chunks.
        wq_sb = consts.tile([128, 2, DIM_HEAD], F32R)
        wk_sb = consts.tile([128, 2, DIM_HEAD], F32R)
        # fmap [256, n] -> [128p, 2, n]
        fk_sb = consts.tile([128, 2, XY], F32R)
        fq_sb = consts.tile([128, 2, QCHUNK], F32R)
        fk_src = fmap_k.rearrange("(a p) n -> p a n", p=128)
        nc.sync.dma_start(out=wq_sb, in_=wqt.rearrange("(a p) d -> p a d", p=128))
        nc.sync.dma_start(out=wk_sb, in_=wkt.rearrange("(a p) d -> p a d", p=128))
        nc.sync.dma_start(out=fq_sb, in_=fmap_q.rearrange("(a p) n -> p a n", p=128))
        # fmap_k in 2 column-halves so the k projection overlaps the load
        nc.sync.dma_start(out=fk_sb[:, :, 0:2048], in_=fk_src[:, :, 0:2048])
        nc.sync.dma_start(out=fk_sb[:, :, 2048:4096], in_=fk_src[:, :, 2048:4096])

        q_sb = consts.tile([128, QCHUNK], QK_DT)  # [d, x] for this core's queries
        k_sb = consts.tile([128, XY], QK_DT)      # [d, uv]

        # ---- q/k projections: out[d, n] = sum_c W^T[c, d] * fmap[c, n] ----
        with tc.tile_pool(name="proj_ps", bufs=2, space="PSUM") as proj_pool:
            ps_q = proj_pool.tile([128, QCHUNK], F32, tag="proj")
            for j in range(QCHUNK // 512):
                sl = slice(j * 512, (j + 1) * 512)
                nc.tensor.matmul(ps_q[:, sl], lhsT=wq_sb[:, 0, :],
                                 rhs=fq_sb[:, 0, sl],
                                 start=True, stop=False)
                nc.tensor.matmul(ps_q[:, sl], lhsT=wq_sb[:, 1, :],
                                 rhs=fq_sb[:, 1, sl],
                                 start=False, stop=True)
            nc.scalar.copy(q_sb, ps_q)

            for h in range(2):
                ps_k = proj_pool.tile([128, QCHUNK], F32, tag="proj")
                for j in range(4):
                    osl = slice(j * 512, (j + 1) * 512)
                    ksl = slice(h * 2048 + j * 512, h * 2048 + (j + 1) * 512)
                    nc.tensor.matmul(ps_k[:, osl], lhsT=wk_sb[:, 0, :],
                                     rhs=fk_sb[:, 0, ksl],
                                     start=True, stop=False)
                    nc.tensor.matmul(ps_k[:, osl], lhsT=wk_sb[:, 1, :],
                                     rhs=fk_sb[:, 1, ksl],
                                     start=False, stop=True)
                nc.scalar.copy(k_sb[:, h * 2048:(h + 1) * 2048], ps_k)

        # ---- scores + softmax, 16 query tiles of 128 ----
        with tc.tile_pool(name="score_ps", bufs=2, space="PSUM") as ps_pool, \
             tc.tile_pool(name="soft", bufs=4) as soft_pool, \
             tc.tile_pool(name="small", bufs=4) as small_pool:
            for qt in range(QCHUNK // 128):
                qsl = q_sb[:, qt * 128:(qt + 1) * 128]
                et = soft_pool.tile([128, XY], F32, tag="et")
                pp = small_pool.tile([128, 2], F32, tag="pp")
                for half in range(2):
                    ps = ps_pool.tile([128, 2048], F32, tag="score")
                    for j in range(4):
                        osl = slice(j * 512, (j + 1) * 512)
                        ksl = slice(half * 2048 + j * 512, half * 2048 + (j + 1) * 512)
                        nc.tensor.matmul(ps[:, osl], lhsT=qsl,
                                         rhs=k_sb[:, ksl],
                                         start=True, stop=True)
                    # exp over 2048 scores/partition straight out of PSUM,
                    # with per-row partial sums accumulated for free.
                    nc.scalar.activation(
                        out=et[:, half * 2048:(half + 1) * 2048], in_=ps,
                        func=mybir.ActivationFunctionType.Exp,
                        accum_out=pp[:, half:half + 1])
                den = small_pool.tile([128, 1], F32, tag="den")
                nc.vector.tensor_add(den, pp[:, 0:1], pp[:, 1:2])
                nc.vector.reciprocal(den, den)
                nc.vector.tensor_scalar_mul(et, et, den)
                nc.sync.dma_start(out=out[qt * 128:(qt + 1) * 128, :], in_=et)


def build_program():
    nc = bacc.Bacc("TRN2", target_bir_lowering=False, debug=False,
                   enable_asserts=False)
    fmap_k = nc.dram_tensor("fmap_k", [C, XY], F32R, kind="ExternalInput").ap()
    fmap_q = nc.dram_tensor("fmap_q", [C, QCHUNK], F32R, kind="ExternalInput").ap()
    wqt = nc.dram_tensor("wqt", [C, DIM_HEAD], F32R, kind="ExternalInput").ap()
    wkt = nc.dram_tensor("wkt", [C, DIM_HEAD], F32R, kind="ExternalInput").ap()
    out = nc.dram_tensor("out", [QCHUNK, XY], F32, kind="ExternalOutput").ap()

    with tile.TileContext(nc) as tc:
        _emit(tc, fmap_k, fmap_q, wqt, wkt, out)
    nc.compile()
    return nc


_CACHE = {}


def _get_nc():
    if "nc" not in _CACHE:
        _CACHE["nc"] = build_program()
    return _CACHE["nc"]


def make_in_maps(fmap, W_qk):
    fm = np.ascontiguousarray(np.asarray(fmap, dtype=np.float32).reshape(C, XY))
    W = np.asarray(W_qk, dtype=np.float32)
    in_maps = []
    for core in range(N_CORES):
        hd, qhalf = divmod(core, 2)
        wq = W[hd * DIM_HEAD:(hd + 1) * DIM_HEAD] * np.float32(SCALE)
        wk = W[HEADS * DIM_HEAD + hd * DIM_HEAD:
               HEADS * DIM_HEAD + (hd + 1) * DIM_HEAD]
        in_maps.append({
            "fmap_k": fm,
            "fmap_q": np.ascontiguousarray(fm[:, qhalf * QCHUNK:(qhalf + 1) * QCHUNK]),
            "wqt": np.ascontiguousarray(wq.T),
            "wkt": np.ascontiguousarray(wk.T),
        })
    return in_maps


def assemble(per_core_outs):
    out = np.empty((HEADS, XY, XY), dtype=np.float32)
    for core in range(N_CORES):
        hd, qhalf = divmod(core, 2)
        out[hd, qhalf * QCHUNK:(qhalf + 1) * QCHUNK, :] = per_core_outs[core]
    return out.reshape(1, HEADS, XY, XY)


def kernel(fmap, W_qk, trace=False):
    nc = _get_nc()
    in_maps = make_in_maps(fmap, W_qk)
    res = bass_utils.run_bass_kernel_spmd(
        nc, in_maps, core_ids=list(range(N_CORES)), trace=trace)
    out = assemble([res.results[c]["out"] for c in range(N_CORES)])
    if trace:
        kernel.last_exec_time_ns = res.exec_time_ns
        kernel.last_results = res
    return out


# revision 7
# speedup vs baseline: 1.0484x; 1.0484x over previous
"""Trainium2 Bass kernel for 4-head spatial attention score softmax.

Reference computation:
    qk = einsum('bcxy,oc->boxy', fmap[1,256,64,64], W_qk[1024,256])
    q, k = split(qk, 2, axis=1)             # each [1, 512, 64, 64]
    q = q reshaped to heads, scaled by 128^-0.5
    sim[b,h,xy,uv] = q . k  (contraction over dim_head=128)
    out = softmax(sim, axis=-1)             # [1, 4, 4096, 4096] f32

Sharding: 8 cores = 4 heads x 2 query-halves. Each core projects q for its
2048 query columns + k for all 4096 columns (both via PE matmuls over the
channel dim), computes scores with f32r (FP22) matmuls, softmax
(exp on ScalarE with accumulated row sums, normalize on VectorE), and
streams its [2048, 4096] f32 output slab to HBM.
"""

import numpy as np

import concourse.bacc as bacc
import concourse.mybir as mybir
import concourse.tile as tile
from concourse import bass_utils

HEADS = 4
DIM_HEAD = 128
C = 256          # input channels
XY = 4096        # 64*64 spatial positions
QCHUNK = 2048    # query positions per core
N_CORES = 8
SCALE = DIM_HEAD ** -0.5

F32 = mybir.dt.float32
F32R = mybir.dt.float32r
BF16 = mybir.dt.bfloat16

# dtype of the q/k operands of the big score matmuls. 16-bit halves the PE
# streaming cost vs f32r (4-byte moving operand streams at ~2 cyc/elem) and
# enables fast weight load. fp16 over bf16: q/k are O(1), so the e5m10
# mantissa (exact inside the PE's FP22) cuts quantization error ~8x.
QK_DT = mybir.dt.float16


def _emit(tc, fmap_k, fmap_q, wqt, wkt, out):
    nc = tc.nc

    with tc.tile_pool(name="consts", bufs=1) as consts:
        # Weights transposed on host: [c, d] with c split into 2 partition chunks.
        wq_sb = consts.tile([128, 2, DIM_HEAD], F32R)
        wk_sb = consts.tile([128, 2, DIM_HEAD], F32R)
        # fmap [256, n] -> [128p, 2, n]
        fk_sb = consts.tile([128, 2, XY], F32R)
        fq_sb = consts.tile([128, 2, QCHUNK], F32R)
        warm_sb = consts.tile([128, 512], QK_DT)
        fk_src = fmap_k.rearrange("(a p) n -> p a n", p=128)
        nc.sync.dma_start(out=wq_sb, in_=wqt.rearrange("(a p) d -> p a d", p=128))
        nc.sync.dma_start(out=wk_sb, in_=wkt.rearrange("(a p) d -> p a d", p=128))
        nc.sync.dma_start(out=fq_sb, in_=fmap_q.rearrange("(a p) n -> p a n", p=128))
        # fmap_k in column chunks so the k projection overlaps the load
        KCH = 1024
        for c in range(XY // KCH):
            nc.sync.dma_start(out=fk_sb[:, :, c * KCH:(c + 1) * KCH],
                              in_=fk_src[:, :, c * KCH:(c + 1) * KCH])

        q_sb = consts.tile([128, QCHUNK], QK_DT)  # [d, x] for this core's queries
        k_sb = consts.tile([128, XY], QK_DT)      # [d, uv]

        nc.vector.memset(warm_sb, 0.0)

        # ---- q/k projections: out[d, n] = sum_c W^T[c, d] * fmap[c, n] ----
        with tc.tile_pool(name="proj_ps", bufs=2, space="PSUM") as proj_pool:
            # PE warmup: dummy matmuls with no load deps keep TensorE busy
            # through the input-DMA window, so the HAM clock gate is at
            # 2.4 GHz by the time real matmuls arrive (cold PE at startup
            # was the serialization bottleneck).
            warm_ps = proj_pool.tile([128, KCH], F32, tag="proj")
            for i in range(48):
                nc.tensor.matmul(warm_ps[:, 0:512], lhsT=warm_sb[:, 0:128],
                                 rhs=warm_sb, start=True, stop=True)

            for cq in range(QCHUNK // KCH):
                ps_q = proj_pool.tile([128, KCH], F32, tag="proj")
                for j in range(KCH // 512):
                    osl = slice(j * 512, (j + 1) * 512)
                    qsl2 = slice(cq * KCH + j * 512, cq * KCH + (j + 1) * 512)
                    nc.tensor.matmul(ps_q[:, osl], lhsT=wq_sb[:, 0, :],
                                     rhs=fq_sb[:, 0, qsl2],
                                     start=True, stop=False)
                    nc.tensor.matmul(ps_q[:, osl], lhsT=wq_sb[:, 1, :],
                                     rhs=fq_sb[:, 1, qsl2],
                                     start=False, stop=True)
                nc.scalar.copy(q_sb[:, cq * KCH:(cq + 1) * KCH], ps_q)

            for c in range(XY // KCH):
                ps_k = proj_pool.tile([128, KCH], F32, tag="proj")
                for j in range(KCH // 512):
                    osl = slice(j * 512, (j + 1) * 512)
                    ksl = slice(c * KCH + j * 512, c * KCH + (j + 1) * 512)
                    nc.tensor.matmul(ps_k[:, osl], lhsT=wk_sb[:, 0, :],
                                     rhs=fk_sb[:, 0, ksl],
                                     start=True, stop=False)
                    nc.tensor.matmul(ps_k[:, osl], lhsT=wk_sb[:, 1, :],
                                     rhs=fk_sb[:, 1, ksl],
                                     start=False, stop=True)
                nc.scalar.copy(k_sb[:, c * KCH:(c + 1) * KCH], ps_k)

        # ---- scores + softmax, 16 query tiles of 128 ----
        with tc.tile_pool(name="score_ps", bufs=2, space="PSUM") as ps_pool, \
             tc.tile_pool(name="soft", bufs=4) as soft_pool, \
             tc.tile_pool(name="small", bufs=4) as small_pool:
            for qt in range(QCHUNK // 128):
                qsl = q_sb[:, qt * 128:(qt + 1) * 128]
                et = soft_pool.tile([128, XY], F32, tag="et")
                pp = small_pool.tile([128, 2], F32, tag="pp")
                for half in range(2):
                    ps = ps_pool.tile([128, 2048], F32, tag="score")
                    for j in range(4):
                        osl = slice(j * 512, (j + 1) * 512)
                        ksl = slice(half * 2048 + j * 512, half * 2048 + (j + 1) * 512)
                        nc.tensor.matmul(ps[:, osl], lhsT=qsl,
                                         rhs=k_sb[:, ksl],
                                         start=True, stop=True)
                    # exp over 2048 scores/partition straight out of PSUM,
                    # with per-row partial sums accumulated for free.
                    nc.scalar.activation(
                        out=et[:, half * 2048:(half + 1) * 2048], in_=ps,
                        func=mybir.ActivationFunctionType.Exp,
                        accum_out=pp[:, half:half + 1])
                den = small_pool.tile([128, 1], F32, tag="den")
                nc.vector.tensor_add(den, pp[:, 0:1], pp[:, 1:2])
                nc.vector.reciprocal(den, den)
                nc.vector.tensor_scalar_mul(et, et, den)
                nc.sync.dma_start(out=out[qt * 128:(qt + 1) * 128, :], in_=et)


def build_program():
    nc = bacc.Bacc("TRN2", target_bir_lowering=False, debug=False,
                   enable_asserts=False)
    fmap_k = nc.dram_tensor("fmap_k", [C, XY], F32R, kind="ExternalInput").ap()
    fmap_q = nc.dram_tensor("fmap_q", [C, QCHUNK], F32R, kind="ExternalInput").ap()
    wqt = nc.dram_tensor("wqt", [C, DIM_HEAD], F32R, kind="ExternalInput").ap()
    wkt = nc.dram_tensor("wkt", [C, DIM_HEAD], F32R, kind="ExternalInput").ap()
    out = nc.dram_tensor("out", [QCHUNK, XY], F32, kind="ExternalOutput").ap()

    with tile.TileContext(nc) as tc:
        _emit(tc, fmap_k, fmap_q, wqt, wkt, out)
    nc.compile()
    return nc


_CACHE = {}


def _get_nc():
    if "nc" not in _CACHE:
        _CACHE["nc"] = build_program()
    return _CACHE["nc"]


def make_in_maps(fmap, W_qk):
    fm = np.ascontiguousarray(np.asarray(fmap, dtype=np.float32).reshape(C, XY))
    W = np.asarray(W_qk, dtype=np.float32)
    in_maps = []
    for core in range(N_CORES):
        hd, qhalf = divmod(core, 2)
        wq = W[hd * DIM_HEAD:(hd + 1) * DIM_HEAD] * np.float32(SCALE)
        wk = W[HEADS * DIM_HEAD + hd * DIM_HEAD:
               HEADS * DIM_HEAD + (hd + 1) * DIM_HEAD]
        in_maps.append({
            "fmap_k": fm,
            "fmap_q": np.ascontiguousarray(fm[:, qhalf * QCHUNK:(qhalf + 1) * QCHUNK]),
            "wqt": np.ascontiguousarray(wq.T),
            "wkt": np.ascontiguousarray(wk.T),
        })
    return in_maps


def assemble(per_core_outs):
    out = np.empty((HEADS, XY, XY), dtype=np.float32)
    for core in range(N_CORES):
        hd, qhalf = divmod(core, 2)
        out[hd, qhalf * QCHUNK:(qhalf + 1) * QCHUNK, :] = per_core_outs[core]
    return out.reshape(1, HEADS, XY, XY)


def kernel(fmap, W_qk, trace=False):
    nc = _get_nc()
    in_maps = make_in_maps(fmap, W_qk)
    res = bass_utils.run_bass_kernel_spmd(
        nc, in_maps, core_ids=list(range(N_CORES)), trace=trace)
    out = assemble([res.results[c]["out"] for c in range(N_CORES)])
    if trace:
        kernel.last_exec_time_ns = res.exec_time_ns
        kernel.last_results = res
    return out


# revision 10
# speedup vs baseline: 1.1380x; 1.0854x over previous
"""Trainium2 Bass kernel for 4-head spatial attention score softmax.

Reference computation:
    qk = einsum('bcxy,oc->boxy', fmap[1,256,64,64], W_qk[1024,256])
    q, k = split(qk, 2, axis=1)             # each [1, 512, 64, 64]
    q = q reshaped to heads, scaled by 128^-0.5
    sim[b,h,xy,uv] = q . k  (contraction over dim_head=128)
    out = softmax(sim, axis=-1)             # [1, 4, 4096, 4096] f32

Sharding: 8 cores = 4 heads x 2 query-halves. Each core projects q for its
2048 query columns + k for all 4096 columns (both via PE matmuls over the
channel dim), computes scores with f32r (FP22) matmuls, softmax
(exp on ScalarE with accumulated row sums, normalize on VectorE), and
streams its [2048, 4096] f32 output slab to HBM.
"""

import numpy as np

import concourse.bacc as bacc
import concourse.mybir as mybir
import concourse.tile as tile
from concourse import bass_utils

HEADS = 4
DIM_HEAD = 128
C = 256          # input channels
XY = 4096        # 64*64 spatial positions
QCHUNK = 2048    # query positions per core
N_CORES = 8
SCALE = DIM_HEAD ** -0.5

F32 = mybir.dt.float32
F32R = mybir.dt.float32r
BF16 = mybir.dt.bfloat16

# dtype of the q/k operands of the big score matmuls. 16-bit halves the PE
# streaming cost vs f32r (4-byte moving operand streams at ~2 cyc/elem) and
# enables fast weight load. fp16 over bf16: q/k are O(1), so the e5m10
# mantissa (exact inside the PE's FP22) cuts quantization error ~8x.
QK_DT = mybir.dt.float16


def _emit(tc, fmap_k, fmap_q, wqt, wkt, out):
    nc = tc.nc

    with tc.tile_pool(name="consts", bufs=1) as consts:
        # Weights transposed on host: [c, d] with c split into 2 partition chunks.
        wq_sb = consts.tile([128, 2, DIM_HEAD], F32R)
        wk_sb = consts.tile([128, 2, DIM_HEAD], F32R)
        # fmap [256, n] -> [128p, 2, n]
        fk_sb = consts.tile([128, 2, XY], F32R)
        fq_sb = consts.tile([128, 2, QCHUNK], F32R)
        warm_sb = consts.tile([128, 512], QK_DT)
        fk_src = fmap_k.rearrange("(a p) n -> p a n", p=128)
        nc.sync.dma_start(out=wq_sb, in_=wqt.rearrange("(a p) d -> p a d", p=128))
        nc.sync.dma_start(out=wk_sb, in_=wkt.rearrange("(a p) d -> p a d", p=128))
        nc.sync.dma_start(out=fq_sb, in_=fmap_q.rearrange("(a p) n -> p a n", p=128))
        # fmap_k in column chunks so the k projection overlaps the load
        KCH = 1024
        for c in range(XY // KCH):
            nc.sync.dma_start(out=fk_sb[:, :, c * KCH:(c + 1) * KCH],
                              in_=fk_src[:, :, c * KCH:(c + 1) * KCH])

        q_sb = consts.tile([128, QCHUNK], QK_DT)  # [d, x] for this core's queries
        k_sb = consts.tile([128, XY], QK_DT)      # [d, uv]

        nc.vector.memset(warm_sb, 0.0)

        # ---- q/k projections: out[d, n] = sum_c W^T[c, d] * fmap[c, n] ----
        with tc.tile_pool(name="proj_ps", bufs=2, space="PSUM") as proj_pool:
            # PE warmup: dummy matmuls with no load deps keep TensorE busy
            # through the input-DMA window, so the HAM clock gate is at
            # 2.4 GHz by the time real matmuls arrive (cold PE at startup
            # was the serialization bottleneck).
            warm_ps = proj_pool.tile([128, KCH], F32, tag="proj")
            for i in range(14):
                nc.tensor.matmul(warm_ps[:, 0:512], lhsT=warm_sb[:, 0:128],
                                 rhs=warm_sb, start=True, stop=True)

            for cq in range(QCHUNK // KCH):
                ps_q = proj_pool.tile([128, KCH], F32, tag="proj")
                for j in range(KCH // 512):
                    osl = slice(j * 512, (j + 1) * 512)
                    qsl2 = slice(cq * KCH + j * 512, cq * KCH + (j + 1) * 512)
                    nc.tensor.matmul(ps_q[:, osl], lhsT=wq_sb[:, 0, :],
                                     rhs=fq_sb[:, 0, qsl2],
                                     start=True, stop=False)
                    nc.tensor.matmul(ps_q[:, osl], lhsT=wq_sb[:, 1, :],
                                     rhs=fq_sb[:, 1, qsl2],
                                     start=False, stop=True)
                nc.scalar.copy(q_sb[:, cq * KCH:(cq + 1) * KCH], ps_q)

            for c in range(XY // KCH):
                ps_k = proj_pool.tile([128, KCH], F32, tag="proj")
                for j in range(KCH // 512):
                    osl = slice(j * 512, (j + 1) * 512)
                    ksl = slice(c * KCH + j * 512, c * KCH + (j + 1) * 512)
                    nc.tensor.matmul(ps_k[:, osl], lhsT=wk_sb[:, 0, :],
                                     rhs=fk_sb[:, 0, ksl],
                                     start=True, stop=False)
                    nc.tensor.matmul(ps_k[:, osl], lhsT=wk_sb[:, 1, :],
                                     rhs=fk_sb[:, 1, ksl],
                                     start=False, stop=True)
                nc.scalar.copy(k_sb[:, c * KCH:(c + 1) * KCH], ps_k)

        # ---- scores + softmax, 16 query tiles of 128 ----
        with tc.tile_pool(name="score_ps", bufs=2, space="PSUM") as ps_pool, \
             tc.tile_pool(name="soft", bufs=4) as soft_pool, \
             tc.tile_pool(name="small", bufs=4) as small_pool:
            for qt in range(QCHUNK // 128):
                qsl = q_sb[:, qt * 128:(qt + 1) * 128]
                et = soft_pool.tile([128, XY], F32, tag="et")
                pp = small_pool.tile([128, 2], F32, tag="pp")
                for half in range(2):
                    ps = ps_pool.tile([128, 2048], F32, tag="score")
                    for j in range(4):
                        osl = slice(j * 512, (j + 1) * 512)
                        ksl = slice(half * 2048 + j * 512, half * 2048 + (j + 1) * 512)
                        nc.tensor.matmul(ps[:, osl], lhsT=qsl,
                                         rhs=k_sb[:, ksl],
                                         start=True, stop=True)
                    # exp over 2048 scores/partition straight out of PSUM,
                    # with per-row partial sums accumulated for free.
                    nc.scalar.activation(
                        out=et[:, half * 2048:(half + 1) * 2048], in_=ps,
                        func=mybir.ActivationFunctionType.Exp,
                        accum_out=pp[:, half:half + 1])
                den = small_pool.tile([128, 1], F32, tag="den")
                nc.vector.tensor_add(den, pp[:, 0:1], pp[:, 1:2])
                nc.vector.reciprocal(den, den)
                nc.vector.tensor_scalar_mul(et, et, den)
                nc.sync.dma_start(out=out[qt * 128:(qt + 1) * 128, :], in_=et)


def build_program():
    nc = bacc.Bacc("TRN2", target_bir_lowering=False, debug=False,
                   enable_asserts=False)
    fmap_k = nc.dram_tensor("fmap_k", [C, XY], F32R, kind="ExternalInput").ap()
    fmap_q = nc.dram_tensor("fmap_q", [C, QCHUNK], F32R, kind="ExternalInput").ap()
    wqt = nc.dram_tensor("wqt", [C, DIM_HEAD], F32R, kind="ExternalInput").ap()
    wkt = nc.dram_tensor("wkt", [C, DIM_HEAD], F32R, kind="ExternalInput").ap()
    out = nc.dram_tensor("out", [QCHUNK, XY], F32, kind="ExternalOutput").ap()

    with tile.TileContext(nc) as tc:
        _emit(tc, fmap_k, fmap_q, wqt, wkt, out)
    nc.compile()
    return nc


_CACHE = {}


def _get_nc():
    if "nc" not in _CACHE:
        _CACHE["nc"] = build_program()
    return _CACHE["nc"]


def make_in_maps(fmap, W_qk):
    fm = np.ascontiguousarray(np.asarray(fmap, dtype=np.float32).reshape(C, XY))
    W = np.asarray(W_qk, dtype=np.float32)
    in_maps = []
    for core in range(N_CORES):
        hd, qhalf = divmod(core, 2)
        wq = W[hd * DIM_HEAD:(hd + 1) * DIM_HEAD] * np.float32(SCALE)
        wk = W[HEADS * DIM_HEAD + hd * DIM_HEAD:
               HEADS * DIM_HEAD + (hd + 1) * DIM_HEAD]
        in_maps.append({
            "fmap_k": fm,
            "fmap_q": np.ascontiguousarray(fm[:, qhalf * QCHUNK:(qhalf + 1) * QCHUNK]),
            "wqt": np.ascontiguousarray(wq.T),
            "wkt": np.ascontiguousarray(wk.T),
        })
    return in_maps


def assemble(per_core_outs):
    out = np.empty((HEADS, XY, XY), dtype=np.float32)
    for core in range(N_CORES):
        hd, qhalf = divmod(core, 2)
        out[hd, qhalf * QCHUNK:(qhalf + 1) * QCHUNK, :] = per_core_outs[core]
    return out.reshape(1, HEADS, XY, XY)


def kernel(fmap, W_qk, trace=False):
    nc = _get_nc()
    in_maps = make_in_maps(fmap, W_qk)
    res = bass_utils.run_bass_kernel_spmd(
        nc, in_maps, core_ids=list(range(N_CORES)), trace=trace)
    out = assemble([res.results[c]["out"] for c in range(N_CORES)])
    if trace:
        kernel.last_exec_time_ns = res.exec_time_ns
        kernel.last_results = res
    return out
